# revision 28
# baseline (speedup 1.0000x reference)
"""Distributed Trainium2 kernel for AdaptiveSimpleGCNConv.

Math: out = D^{-1/2} (A_set + I) D^{-1/2} @ x @ W.T + b
  A_set: dense 0/1 adjacency from edge_index (duplicates collapse), N=8192.

Strategy (8 NeuronCores, 1D row partition of nodes):
  - Host: dedup edges, compute degree/d=1/sqrt(deg), fold the column scale
    into x' = d*x. Quantize x' to fp8 (hi) plus a 64x-scaled fp8 residual
    (lo). Permute the COLUMN (source-node) space so the columns with the
    largest quantization-error energy come first; the lo correction is only
    applied to the first NLO=36 of 64 column-chunks (~72% of the error
    energy), which keeps the PE cost at 1.56x a single fp8 pass while the
    final relative error stays ~1.3e-2 (< 2e-2 gate).
  - Device k: stream adjacency supertiles (fp8, values 0/1/2 exact); for
    each chunk-pair one fp8 DoubleRow matmul (2 contraction chunks per
    instruction, 2 elem/cycle) accumulates y_hi per 512-row window, plus a
    second DoubleRow matmul into y_lo for the corrected chunk range.
    Epilogue: cast y_hi/y_lo to bf16, out = (y_hi@W.T + y_lo@(W/64).T)*d + b
    via two accumulating PSUM matmuls, stored bf16 in a [part, group, feat]
    layout the host untangles (and casts back to fp32).
  - DMA plan: big transfers, few instructions (issue costs ~0.6us each and
    only 8 can be in flight): x hi+lo planes packed into one DRAM tensor
    split in 4 pieces, interleaved with 9 adjacency supertile DMAs on the
    sync queue in exact consumption order; consts + outputs on the scalar
    queue. The last two supertiles are half-size to shorten the drain tail.
  - No collectives: x planes are replicated to every core by the host.
"""

import sys

sys.path.insert(0, "/opt/trn_rl_repo")

import numpy as np
import ml_dtypes

N = 8192
D = 128
NCORES = 8
RPC = N // NCORES   # 1024 rows per core
NCHUNK = N // 128   # 64 contraction chunks
NPAIR = NCHUNK // 2  # 32 DoubleRow chunk-pairs
NLO = 32            # chunks receiving the lo correction
NLO_PAIR = NLO // 2
NWIN = RPC // 512   # 2 row windows per core
# bulk adjacency supertiles on the sync queue (chunks 0-59); the final 4
# chunks ("tail" supertile) ride the scalar queue EARLY so their completion
# is not gated by the slowest DMA engine draining the whole bulk queue.
SUPERS = [4, 8, 8, 8, 8, 8, 8, 6, 2]    # sync-queue supertiles, chunks 0-59
TAIL_C0, TAIL_SZ = 60, 4                # early scalar-queue supertile
# packed x pieces: (hi chunk range, lo chunk range); xp4 rides scalar early
XP_HI = [(0, 8), (8, 16), (16, 32), (32, 48), (48, 64)]
XP_LO = [(0, 8), (8, 16), (16, 32), (32, 32), (32, 32)]
XP_LENS = [(h1 - h0) + (l1 - l0)
           for (h0, h1), (l0, l1) in zip(XP_HI, XP_LO)]
XP_STARTS = [sum(XP_LENS[:i]) for i in range(len(XP_LENS))]
XTOT = sum(XP_LENS)  # 96 chunks
# sync-queue x pieces land before these chunk offsets (xp4 is on scalar)
XP_BEFORE_CHUNK = [0, 4, 12, 28, None]
N_WARM = 9          # dummy matmuls to ramp the PE clock during DMA wait
S_LO = 64.0         # scale for the lo fp8 plane
BF16 = ml_dtypes.bfloat16
FP8 = ml_dtypes.float8_e4m3fn

_CACHE = {}


def _build_nc():
    from concourse import bacc, bass, tile, mybir

    adt = mybir.dt.float8e4

    nc = bacc.Bacc("TRN2", target_bir_lowering=False, debug=False,
                   num_devices=NCORES)

    adjt_ext = nc.declare_dram_parameter(
        "adjT", [128, NCHUNK, RPC], adt, isOutput=False)
    xc_ext = nc.declare_dram_parameter(
        "xc", [128, XTOT, D], adt, isOutput=False)
    wt_ext = nc.declare_dram_parameter(
        "wT", [D, D], mybir.dt.bfloat16, isOutput=False)
    wt64_ext = nc.declare_dram_parameter(
        "wT64", [D, D], mybir.dt.bfloat16, isOutput=False)
    bb_ext = nc.declare_dram_parameter(
        "bb", [128, D], mybir.dt.float32, isOutput=False)
    dr_ext = nc.declare_dram_parameter(
        "dr", [128, RPC // 128], mybir.dt.float32, isOutput=False)
    out_ext = nc.declare_dram_parameter(
        "out", [128, RPC // 128, D], mybir.dt.bfloat16, isOutput=True)

    DR = mybir.MatmulPerfMode.DoubleRow

    with tile.TileContext(nc) as tc:
        with (
            tc.tile_pool(name="const", bufs=1) as constp,
            tc.tile_pool(name="adj", bufs=1) as adjp,
            tc.tile_pool(name="yt", bufs=2) as ytp,
            tc.tile_pool(name="ot", bufs=2) as otp,
            tc.tile_pool(name="ps_y", bufs=1, space=bass.MemorySpace.PSUM) as psy,
            tc.tile_pool(name="ps_o", bufs=2, space=bass.MemorySpace.PSUM) as pso,
        ):
            xp = [constp.tile([128, XP_LENS[i], D], adt, name=f"xp{i}",
                              tag=f"xp{i}") for i in range(len(XP_LENS))]

            def load_xpiece(i):
                nc.sync.dma_start(
                    out=xp[i][:],
                    in_=xc_ext[:, XP_STARTS[i]:XP_STARTS[i] + XP_LENS[i], :])

            def xsl_hi(q):
                c0 = 2 * q
                i = next(k for k, (h0, h1) in enumerate(XP_HI)
                         if h0 <= c0 < h1)
                o = c0 - XP_HI[i][0]
                return xp[i][:, o:o + 2, :]

            def xsl_lo(q):
                c0 = 2 * q
                i = next(k for k, (l0, l1) in enumerate(XP_LO)
                         if l0 <= c0 < l1)
                o = (XP_HI[i][1] - XP_HI[i][0]) + (c0 - XP_LO[i][0])
                return xp[i][:, o:o + 2, :]

            wt = constp.tile([D, D], mybir.dt.bfloat16, name="wt")
            nc.scalar.dma_start(out=wt[:], in_=wt_ext[:])
            wt64 = constp.tile([D, D], mybir.dt.bfloat16, name="wt64")
            nc.scalar.dma_start(out=wt64[:], in_=wt64_ext[:])
            bb = constp.tile([128, D], mybir.dt.float32, name="bb")
            nc.scalar.dma_start(out=bb[:], in_=bb_ext[:])
            dr = constp.tile([128, RPC // 128], mybir.dt.float32, name="dr")
            nc.scalar.dma_start(out=dr[:], in_=dr_ext[:])

            ps_hi = [psy.tile([128, 512], mybir.dt.float32, tag=f"pshi{w}",
                              name=f"ps_hi{w}") for w in range(NWIN)]
            ps_lo = [psy.tile([128, 512], mybir.dt.float32, tag=f"pslo{w}",
                              name=f"ps_lo{w}") for w in range(NWIN)]

            # warmup: ramp the PE clock out of its low p-state while the
            # first DMAs are in flight (the real stream then runs full speed)
            scr = constp.tile([128, 512], adt, name="warm_src")
            nc.gpsimd.memset(scr[:], 0)
            ps_w = psy.tile([128, 512], mybir.dt.float32, tag="pswarm",
                            name="ps_warm")
            for _ in range(N_WARM):
                nc.tensor.matmul(ps_w[:], lhsT=scr[:, :128], rhs=scr[:],
                                 start=True, stop=True)

            # tail supertile (last 4 chunks) + its x piece on the scalar
            # queue, issued up-front: they complete mid-run, so the final
            # accumulation is gated by st7, not by the bulk queue's last
            # bytes draining on the slowest DMA engine.
            nc.scalar.dma_start(
                out=xp[4][:],
                in_=xc_ext[:, XP_STARTS[4]:XP_STARTS[4] + XP_LENS[4], :])
            at_tail = adjp.tile([128, TAIL_SZ, RPC], adt, tag="adjtail",
                                name="adjtail")
            nc.scalar.dma_start(
                out=at_tail[:], in_=adjt_ext[:, TAIL_C0:TAIL_C0 + TAIL_SZ, :])

            def mm(q, j, w, at, start, stop):
                cs = slice(2 * j, 2 * j + 2)
                ws = slice(w * 512, (w + 1) * 512)
                nc.tensor.matmul(
                    ps_hi[w][:],
                    lhsT=xsl_hi(q),
                    rhs=at[:, cs, ws],
                    start=start,
                    stop=stop,
                    perf_mode=DR,
                )
                if q < NLO_PAIR:
                    nc.tensor.matmul(
                        ps_lo[w][:],
                        lhsT=xsl_lo(q),
                        rhs=at[:, cs, ws],
                        start=start,
                        stop=(q == NLO_PAIR - 1),
                        perf_mode=DR,
                    )

            c0 = 0
            for s, sz in enumerate(SUPERS):
                for i, bc in enumerate(XP_BEFORE_CHUNK):
                    if bc == c0:
                        load_xpiece(i)
                at = adjp.tile([128, sz, RPC], adt, tag=f"adj{s}",
                               name=f"adj{s}")
                nc.sync.dma_start(
                    out=at[:], in_=adjt_ext[:, c0:c0 + sz, :])
                q0 = c0 // 2
                if s < len(SUPERS) - 1:
                    for j in range(sz // 2):
                        for w in range(NWIN):
                            mm(q0 + j, j, w, at, start=(q0 + j == 0),
                               stop=False)
                else:
                    # last emitted supertile window-major: window 0 finishes
                    # early so its epilogue overlaps window 1's tail matmuls
                    for w in range(NWIN):
                        for j in range(sz // 2):
                            mm(q0 + j, j, w, at, start=False,
                               stop=(j == sz // 2 - 1))
                c0 += sz
                if s == 2:
                    # tail chunks (arrived early on the scalar queue) are
                    # accumulated mid-stream; PSUM order is free
                    qt = TAIL_C0 // 2
                    for j in range(TAIL_SZ // 2):
                        for w in range(NWIN):
                            mm(qt + j, j, w, at_tail, start=False,
                               stop=False)

            # epilogue: lo copies unblock early (lo accumulation stops at
            # chunk 32); yh copies split across vector/scalar so both
            # windows' chains run in parallel after the final matmuls
            yl0 = ytp.tile([128, 512], mybir.dt.bfloat16, tag="yl0")
            nc.scalar.copy(yl0[:], ps_lo[0][:])
            yl1 = ytp.tile([128, 512], mybir.dt.bfloat16, tag="yl1")
            nc.scalar.copy(yl1[:], ps_lo[1][:])
            yls = [yl0, yl1]
            for w in range(NWIN):
                yh = ytp.tile([128, 512], mybir.dt.bfloat16, tag=f"yh{w}")
                if w == 0:
                    # sliced cast: each slice feeds its W-matmul sooner
                    for m in range(4):
                        sl = slice(m * 128, (m + 1) * 128)
                        nc.vector.tensor_copy(yh[:, sl], ps_hi[w][:, sl])
                else:
                    nc.scalar.copy(yh[:], ps_hi[w][:])
                yl = yls[w]
                ot = otp.tile([128, 4, D], mybir.dt.bfloat16, tag="outtile")
                ps_o = pso.tile([128, 4, D], mybir.dt.float32)
                for m in range(4):
                    g = w * 4 + m
                    sl = slice(m * 128, (m + 1) * 128)
                    nc.tensor.matmul(
                        ps_o[:, m, :],
                        lhsT=yh[:, sl],
                        rhs=wt[:],
                        start=True,
                        stop=False,
                    )
                    nc.tensor.matmul(
                        ps_o[:, m, :],
                        lhsT=yl[:, sl],
                        rhs=wt64[:],
                        start=False,
                        stop=True,
                    )
                    nc.vector.scalar_tensor_tensor(
                        out=ot[:, m, :],
                        in0=ps_o[:, m, :],
                        scalar=dr[:, g:g + 1],
                        in1=bb[:],
                        op0=mybir.AluOpType.mult,
                        op1=mybir.AluOpType.add,
                    )
                nc.scalar.dma_start(out=out_ext[:, w * 4:(w + 1) * 4, :],
                                    in_=ot[:])
    nc.compile()
    return nc


def _host_prep(x, edge_index, W, b):
    r = np.asarray(edge_index[0]).astype(np.int64)
    c = np.asarray(edge_index[1]).astype(np.int64)
    uniq = np.unique(r * N + c)
    r_u = uniq // N
    c_u = uniq % N

    degree = np.bincount(r_u, minlength=N).astype(np.float64) + 1.0
    d = (1.0 / np.sqrt(degree)).astype(np.float32)

    xp = np.asarray(x, dtype=np.float32) * d[:, None]
    xh8 = xp.astype(FP8)
    lo = xp - xh8.astype(np.float32)
    xl8 = (lo * S_LO).astype(FP8)

    # permute the column space so the columns with the largest fp8
    # quantization-error energy land in the corrected chunk range [0, NLO)
    order = np.argsort(-(lo * lo).sum(axis=1), kind="stable")
    P = np.empty(N, dtype=np.int64)
    P[order] = np.arange(N)

    def to_chunks(a, nchunk):
        return a.reshape(nchunk, 128, D).transpose(1, 0, 2)  # [128, chk, feat]

    xh_c = to_chunks(xh8[order], NCHUNK)
    xl_c = to_chunks(xl8[order[:NLO * 128]], NLO)
    # packed pieces: [hi range | lo range] per XP_HI/XP_LO (must match device)
    parts = []
    for (h0, h1), (l0, l1) in zip(XP_HI, XP_LO):
        parts.append(xh_c[:, h0:h1])
        if l1 > l0:
            parts.append(xl_c[:, l0:l1])
    xc = np.ascontiguousarray(np.concatenate(parts, axis=1))

    wt = np.ascontiguousarray(np.asarray(W, dtype=np.float32).T).astype(BF16)
    wt64 = np.ascontiguousarray(
        np.asarray(W, dtype=np.float32).T / S_LO).astype(BF16)
    bb = np.ascontiguousarray(
        np.tile(np.asarray(b, dtype=np.float32)[None, :], (128, 1)))

    in_maps = []
    for k in range(NCORES):
        mask = (r_u // RPC) == k
        rr = r_u[mask] - k * RPC  # local row in [0, RPC)
        cs = P[c_u[mask]]         # permuted global col in [0, N)
        adjt = np.zeros((128, NCHUNK, RPC), dtype=FP8)
        # adjt[p, cc, q] corresponds to adj[row = q (local), col = cc*128+p]
        adjt[cs & 127, cs >> 7, rr] = 1.0
        jj = np.arange(RPC)
        ii = P[k * RPC + jj]  # permuted diag index -> column
        adjt[ii & 127, ii >> 7, jj] += np.ones(RPC, dtype=FP8)
        dr = np.ascontiguousarray(
            d[k * RPC:(k + 1) * RPC].reshape(RPC // 128, 128).T)
        in_maps.append({"adjT": adjt, "xc": xc,
                        "wT": wt, "wT64": wt64, "bb": bb, "dr": dr})
    return in_maps


def _gather(res):
    outs = []
    for k in range(NCORES):
        o = np.asarray(res.results[k]["out"])  # [128, RPC//128, D] bf16
        outs.append(o.transpose(1, 0, 2).reshape(RPC, D))
    return np.ascontiguousarray(np.concatenate(outs, axis=0).astype(np.float32))


def kernel(x, edge_index, W, b):
    from concourse.bass_utils import run_bass_kernel_spmd

    in_maps = _host_prep(x, edge_index, W, b)
    if "nc" not in _CACHE:
        _CACHE["nc"] = _build_nc()
    nc = _CACHE["nc"]
    res = run_bass_kernel_spmd(nc, in_maps, core_ids=list(range(NCORES)))
    return _gather(res)


if __name__ == "__main__":
    rng = np.random.default_rng(0)
    x = rng.standard_normal((N, D), dtype=np.float32)
    ei = rng.integers(0, N, size=(2, 262144)).astype(np.int64)
    W = rng.standard_normal((D, D), dtype=np.float32) / np.sqrt(D)
    b = rng.standard_normal(D, dtype=np.float32) * 0.01
    out = kernel(x=x, edge_index=ei, W=W, b=b)
    print(out.shape, out.dtype, float(np.abs(out).mean()))


# revision 31
# speedup vs baseline: 1.0015x; 1.0015x over previous
"""Distributed Trainium2 kernel for AdaptiveSimpleGCNConv.

Math: out = D^{-1/2} (A_set + I) D^{-1/2} @ x @ W.T + b
  A_set: dense 0/1 adjacency from edge_index (duplicates collapse), N=8192.

Strategy (8 NeuronCores, 1D row partition of nodes):
  - Host: dedup edges, compute degree/d=1/sqrt(deg), fold the column scale
    into x' = d*x. Quantize x' to fp8 (hi) plus a 64x-scaled fp8 residual
    (lo). Permute the COLUMN (source-node) space so the columns with the
    largest quantization-error energy come first; the lo correction is only
    applied to the first NLO=36 of 64 column-chunks (~72% of the error
    energy), which keeps the PE cost at 1.56x a single fp8 pass while the
    final relative error stays ~1.3e-2 (< 2e-2 gate).
  - Device k: stream adjacency supertiles (fp8, values 0/1/2 exact); for
    each chunk-pair one fp8 DoubleRow matmul (2 contraction chunks per
    instruction, 2 elem/cycle) accumulates y_hi per 512-row window, plus a
    second DoubleRow matmul into y_lo for the corrected chunk range.
    Epilogue: cast y_hi/y_lo to bf16, out = (y_hi@W.T + y_lo@(W/64).T)*d + b
    via two accumulating PSUM matmuls, stored bf16 in a [part, group, feat]
    layout the host untangles (and casts back to fp32).
  - DMA plan: big transfers, few instructions (issue costs ~0.6us each and
    only 8 can be in flight): x hi+lo planes packed into one DRAM tensor
    split in 4 pieces, interleaved with 9 adjacency supertile DMAs on the
    sync queue in exact consumption order; consts + outputs on the scalar
    queue. The last two supertiles are half-size to shorten the drain tail.
  - No collectives: x planes are replicated to every core by the host.
"""

import sys

sys.path.insert(0, "/opt/trn_rl_repo")

import numpy as np
import ml_dtypes

N = 8192
D = 128
NCORES = 8
RPC = N // NCORES   # 1024 rows per core
NCHUNK = N // 128   # 64 contraction chunks
NPAIR = NCHUNK // 2  # 32 DoubleRow chunk-pairs
NLO = 32            # chunks receiving the lo correction
NLO_PAIR = NLO // 2
NWIN = RPC // 512   # 2 row windows per core
# bulk adjacency supertiles on the sync queue (chunks 0-59); the final 4
# chunks ("tail" supertile) ride the scalar queue EARLY so their completion
# is not gated by the slowest DMA engine draining the whole bulk queue.
SUPERS = [4, 8, 8, 8, 8, 8, 8, 8]       # sync-queue supertiles, chunks 0-59
TAIL_C0, TAIL_SZ = 60, 4                # early scalar-queue supertile
# packed x pieces: (hi chunk range, lo chunk range); xp4 rides scalar early
XP_HI = [(0, 8), (8, 16), (16, 32), (32, 48), (48, 64)]
XP_LO = [(0, 8), (8, 16), (16, 32), (32, 32), (32, 32)]
XP_LENS = [(h1 - h0) + (l1 - l0)
           for (h0, h1), (l0, l1) in zip(XP_HI, XP_LO)]
XP_STARTS = [sum(XP_LENS[:i]) for i in range(len(XP_LENS))]
XTOT = sum(XP_LENS)  # 96 chunks
# sync-queue x pieces land before these chunk offsets (xp4 is on scalar)
XP_BEFORE_CHUNK = [0, 4, 12, 28, None]
N_WARM = 9          # dummy matmuls to ramp the PE clock during DMA wait
S_LO = 64.0         # scale for the lo fp8 plane
BF16 = ml_dtypes.bfloat16
FP8 = ml_dtypes.float8_e4m3fn

_CACHE = {}


def _build_nc():
    from concourse import bacc, bass, tile, mybir

    adt = mybir.dt.float8e4

    nc = bacc.Bacc("TRN2", target_bir_lowering=False, debug=False,
                   num_devices=NCORES)

    adjt_ext = nc.declare_dram_parameter(
        "adjT", [128, NCHUNK, RPC], adt, isOutput=False)
    xc_ext = nc.declare_dram_parameter(
        "xc", [128, XTOT, D], adt, isOutput=False)
    wt_ext = nc.declare_dram_parameter(
        "wT", [D, D], mybir.dt.bfloat16, isOutput=False)
    wt64_ext = nc.declare_dram_parameter(
        "wT64", [D, D], mybir.dt.bfloat16, isOutput=False)
    bb_ext = nc.declare_dram_parameter(
        "bb", [128, D], mybir.dt.float32, isOutput=False)
    dr_ext = nc.declare_dram_parameter(
        "dr", [128, RPC // 128], mybir.dt.float32, isOutput=False)
    out_ext = nc.declare_dram_parameter(
        "out", [128, RPC // 128, D], mybir.dt.bfloat16, isOutput=True)

    DR = mybir.MatmulPerfMode.DoubleRow

    with tile.TileContext(nc) as tc:
        with (
            tc.tile_pool(name="const", bufs=1) as constp,
            tc.tile_pool(name="adj", bufs=1) as adjp,
            tc.tile_pool(name="yt", bufs=2) as ytp,
            tc.tile_pool(name="ot", bufs=2) as otp,
            tc.tile_pool(name="ps_y", bufs=1, space=bass.MemorySpace.PSUM) as psy,
            tc.tile_pool(name="ps_o", bufs=3, space=bass.MemorySpace.PSUM) as pso,
        ):
            xp = [constp.tile([128, XP_LENS[i], D], adt, name=f"xp{i}",
                              tag=f"xp{i}") for i in range(len(XP_LENS))]

            def load_xpiece(i):
                nc.sync.dma_start(
                    out=xp[i][:],
                    in_=xc_ext[:, XP_STARTS[i]:XP_STARTS[i] + XP_LENS[i], :])

            def xsl_hi(q):
                c0 = 2 * q
                i = next(k for k, (h0, h1) in enumerate(XP_HI)
                         if h0 <= c0 < h1)
                o = c0 - XP_HI[i][0]
                return xp[i][:, o:o + 2, :]

            def xsl_lo(q):
                c0 = 2 * q
                i = next(k for k, (l0, l1) in enumerate(XP_LO)
                         if l0 <= c0 < l1)
                o = (XP_HI[i][1] - XP_HI[i][0]) + (c0 - XP_LO[i][0])
                return xp[i][:, o:o + 2, :]

            wt = constp.tile([D, D], mybir.dt.bfloat16, name="wt")
            nc.scalar.dma_start(out=wt[:], in_=wt_ext[:])
            wt64 = constp.tile([D, D], mybir.dt.bfloat16, name="wt64")
            nc.scalar.dma_start(out=wt64[:], in_=wt64_ext[:])
            bb = constp.tile([128, D], mybir.dt.float32, name="bb")
            nc.scalar.dma_start(out=bb[:], in_=bb_ext[:])
            dr = constp.tile([128, RPC // 128], mybir.dt.float32, name="dr")
            nc.scalar.dma_start(out=dr[:], in_=dr_ext[:])

            ps_hi = [psy.tile([128, 512], mybir.dt.float32, tag=f"pshi{w}",
                              name=f"ps_hi{w}") for w in range(NWIN)]
            ps_lo = [psy.tile([128, 512], mybir.dt.float32, tag=f"pslo{w}",
                              name=f"ps_lo{w}") for w in range(NWIN)]

            # warmup: ramp the PE clock out of its low p-state while the
            # first DMAs are in flight (the real stream then runs full speed)
            scr = constp.tile([128, 512], adt, name="warm_src")
            nc.gpsimd.memset(scr[:], 0)
            ps_w = psy.tile([128, 512], mybir.dt.float32, tag="pswarm",
                            name="ps_warm")
            for _ in range(N_WARM):
                nc.tensor.matmul(ps_w[:], lhsT=scr[:, :128], rhs=scr[:],
                                 start=True, stop=True)

            # tail supertile (last 4 chunks) + its x piece on the scalar
            # queue, issued up-front: they complete mid-run, so the final
            # accumulation is gated by st7, not by the bulk queue's last
            # bytes draining on the slowest DMA engine.
            nc.scalar.dma_start(
                out=xp[4][:],
                in_=xc_ext[:, XP_STARTS[4]:XP_STARTS[4] + XP_LENS[4], :])
            at_tail = adjp.tile([128, TAIL_SZ, RPC], adt, tag="adjtail",
                                name="adjtail")
            nc.scalar.dma_start(
                out=at_tail[:], in_=adjt_ext[:, TAIL_C0:TAIL_C0 + TAIL_SZ, :])

            def mm(q, j, w, at, start, stop):
                cs = slice(2 * j, 2 * j + 2)
                ws = slice(w * 512, (w + 1) * 512)
                nc.tensor.matmul(
                    ps_hi[w][:],
                    lhsT=xsl_hi(q),
                    rhs=at[:, cs, ws],
                    start=start,
                    stop=stop,
                    perf_mode=DR,
                )
                if q < NLO_PAIR:
                    nc.tensor.matmul(
                        ps_lo[w][:],
                        lhsT=xsl_lo(q),
                        rhs=at[:, cs, ws],
                        start=start,
                        stop=(q == NLO_PAIR - 1),
                        perf_mode=DR,
                    )

            c0 = 0
            for s, sz in enumerate(SUPERS):
                for i, bc in enumerate(XP_BEFORE_CHUNK):
                    if bc == c0:
                        load_xpiece(i)
                at = adjp.tile([128, sz, RPC], adt, tag=f"adj{s}",
                               name=f"adj{s}")
                nc.sync.dma_start(
                    out=at[:], in_=adjt_ext[:, c0:c0 + sz, :])
                q0 = c0 // 2
                if s < len(SUPERS) - 1:
                    for j in range(sz // 2):
                        for w in range(NWIN):
                            mm(q0 + j, j, w, at, start=(q0 + j == 0),
                               stop=False)
                else:
                    # last emitted supertile window-major: window 0 finishes
                    # early so its epilogue overlaps window 1's tail matmuls
                    for w in range(NWIN):
                        for j in range(sz // 2):
                            mm(q0 + j, j, w, at, start=False,
                               stop=(j == sz // 2 - 1))
                c0 += sz
                if s == 2:
                    # tail chunks (arrived early on the scalar queue) are
                    # accumulated mid-stream; PSUM order is free
                    qt = TAIL_C0 // 2
                    for j in range(TAIL_SZ // 2):
                        for w in range(NWIN):
                            mm(qt + j, j, w, at_tail, start=False,
                               stop=False)

            # epilogue: lo copies unblock early (lo accumulation stops at
            # chunk 32); yh copies split across vector/scalar so both
            # windows' chains run in parallel after the final matmuls
            yl0 = ytp.tile([128, 512], mybir.dt.bfloat16, tag="yl0")
            nc.scalar.copy(yl0[:], ps_lo[0][:])
            yl1 = ytp.tile([128, 512], mybir.dt.bfloat16, tag="yl1")
            nc.scalar.copy(yl1[:], ps_lo[1][:])
            yls = [yl0, yl1]
            for w in range(NWIN):
                yh = ytp.tile([128, 512], mybir.dt.bfloat16, tag=f"yh{w}")
                if w == 0:
                    nc.vector.tensor_copy(yh[:], ps_hi[w][:])
                else:
                    nc.scalar.copy(yh[:], ps_hi[w][:])
                yl = yls[w]
                ot = otp.tile([128, 4, D], mybir.dt.bfloat16, tag="outtile")
                for m in range(4):
                    g = w * 4 + m
                    sl = slice(m * 128, (m + 1) * 128)
                    ps_o = pso.tile([128, D], mybir.dt.float32)
                    nc.tensor.matmul(
                        ps_o[:],
                        lhsT=yh[:, sl],
                        rhs=wt[:],
                        start=True,
                        stop=False,
                    )
                    nc.tensor.matmul(
                        ps_o[:],
                        lhsT=yl[:, sl],
                        rhs=wt64[:],
                        start=False,
                        stop=True,
                    )
                    nc.vector.scalar_tensor_tensor(
                        out=ot[:, m, :],
                        in0=ps_o[:],
                        scalar=dr[:, g:g + 1],
                        in1=bb[:],
                        op0=mybir.AluOpType.mult,
                        op1=mybir.AluOpType.add,
                    )
                    if m == 1:
                        nc.scalar.dma_start(
                            out=out_ext[:, w * 4:w * 4 + 2, :],
                            in_=ot[:, 0:2, :])
                nc.scalar.dma_start(out=out_ext[:, w * 4 + 2:w * 4 + 4, :],
                                    in_=ot[:, 2:4, :])
    nc.compile()
    return nc


def _host_prep(x, edge_index, W, b):
    r = np.asarray(edge_index[0]).astype(np.int64)
    c = np.asarray(edge_index[1]).astype(np.int64)
    uniq = np.unique(r * N + c)
    r_u = uniq // N
    c_u = uniq % N

    degree = np.bincount(r_u, minlength=N).astype(np.float64) + 1.0
    d = (1.0 / np.sqrt(degree)).astype(np.float32)

    xp = np.asarray(x, dtype=np.float32) * d[:, None]
    xh8 = xp.astype(FP8)
    lo = xp - xh8.astype(np.float32)
    xl8 = (lo * S_LO).astype(FP8)

    # permute the column space so the columns with the largest fp8
    # quantization-error energy land in the corrected chunk range [0, NLO)
    order = np.argsort(-(lo * lo).sum(axis=1), kind="stable")
    P = np.empty(N, dtype=np.int64)
    P[order] = np.arange(N)

    def to_chunks(a, nchunk):
        return a.reshape(nchunk, 128, D).transpose(1, 0, 2)  # [128, chk, feat]

    xh_c = to_chunks(xh8[order], NCHUNK)
    xl_c = to_chunks(xl8[order[:NLO * 128]], NLO)
    # packed pieces: [hi range | lo range] per XP_HI/XP_LO (must match device)
    parts = []
    for (h0, h1), (l0, l1) in zip(XP_HI, XP_LO):
        parts.append(xh_c[:, h0:h1])
        if l1 > l0:
            parts.append(xl_c[:, l0:l1])
    xc = np.ascontiguousarray(np.concatenate(parts, axis=1))

    wt = np.ascontiguousarray(np.asarray(W, dtype=np.float32).T).astype(BF16)
    wt64 = np.ascontiguousarray(
        np.asarray(W, dtype=np.float32).T / S_LO).astype(BF16)
    bb = np.ascontiguousarray(
        np.tile(np.asarray(b, dtype=np.float32)[None, :], (128, 1)))

    in_maps = []
    for k in range(NCORES):
        mask = (r_u // RPC) == k
        rr = r_u[mask] - k * RPC  # local row in [0, RPC)
        cs = P[c_u[mask]]         # permuted global col in [0, N)
        adjt = np.zeros((128, NCHUNK, RPC), dtype=FP8)
        # adjt[p, cc, q] corresponds to adj[row = q (local), col = cc*128+p]
        adjt[cs & 127, cs >> 7, rr] = 1.0
        jj = np.arange(RPC)
        ii = P[k * RPC + jj]  # permuted diag index -> column
        adjt[ii & 127, ii >> 7, jj] += np.ones(RPC, dtype=FP8)
        dr = np.ascontiguousarray(
            d[k * RPC:(k + 1) * RPC].reshape(RPC // 128, 128).T)
        in_maps.append({"adjT": adjt, "xc": xc,
                        "wT": wt, "wT64": wt64, "bb": bb, "dr": dr})
    return in_maps


def _gather(res):
    outs = []
    for k in range(NCORES):
        o = np.asarray(res.results[k]["out"])  # [128, RPC//128, D] bf16
        outs.append(o.transpose(1, 0, 2).reshape(RPC, D))
    return np.ascontiguousarray(np.concatenate(outs, axis=0).astype(np.float32))


def kernel(x, edge_index, W, b):
    from concourse.bass_utils import run_bass_kernel_spmd

    in_maps = _host_prep(x, edge_index, W, b)
    if "nc" not in _CACHE:
        _CACHE["nc"] = _build_nc()
    nc = _CACHE["nc"]
    res = run_bass_kernel_spmd(nc, in_maps, core_ids=list(range(NCORES)))
    return _gather(res)


if __name__ == "__main__":
    rng = np.random.default_rng(0)
    x = rng.standard_normal((N, D), dtype=np.float32)
    ei = rng.integers(0, N, size=(2, 262144)).astype(np.int64)
    W = rng.standard_normal((D, D), dtype=np.float32) / np.sqrt(D)
    b = rng.standard_normal(D, dtype=np.float32) * 0.01
    out = kernel(x=x, edge_index=ei, W=W, b=b)
    print(out.shape, out.dtype, float(np.abs(out).mean()))


# revision 33
# speedup vs baseline: 1.0147x; 1.0132x over previous
"""Distributed Trainium2 kernel for AdaptiveSimpleGCNConv.

Math: out = D^{-1/2} (A_set + I) D^{-1/2} @ x @ W.T + b
  A_set: dense 0/1 adjacency from edge_index (duplicates collapse), N=8192.

Strategy (8 NeuronCores, 1D row partition of nodes):
  - Host: dedup edges, compute degree/d=1/sqrt(deg), fold the column scale
    into x' = d*x. Quantize x' to fp8 (hi) plus a 64x-scaled fp8 residual
    (lo). Permute the COLUMN (source-node) space so the columns with the
    largest quantization-error energy come first; the lo correction is only
    applied to the first NLO=36 of 64 column-chunks (~72% of the error
    energy), which keeps the PE cost at 1.56x a single fp8 pass while the
    final relative error stays ~1.3e-2 (< 2e-2 gate).
  - Device k: stream adjacency supertiles (fp8, values 0/1/2 exact); for
    each chunk-pair one fp8 DoubleRow matmul (2 contraction chunks per
    instruction, 2 elem/cycle) accumulates y_hi per 512-row window, plus a
    second DoubleRow matmul into y_lo for the corrected chunk range.
    Epilogue: cast y_hi/y_lo to bf16, out = (y_hi@W.T + y_lo@(W/64).T)*d + b
    via two accumulating PSUM matmuls, stored bf16 in a [part, group, feat]
    layout the host untangles (and casts back to fp32).
  - DMA plan: big transfers, few instructions (issue costs ~0.6us each and
    only 8 can be in flight): x hi+lo planes packed into one DRAM tensor
    split in 4 pieces, interleaved with 9 adjacency supertile DMAs on the
    sync queue in exact consumption order; consts + outputs on the scalar
    queue. The last two supertiles are half-size to shorten the drain tail.
  - No collectives: x planes are replicated to every core by the host.
"""

import sys

sys.path.insert(0, "/opt/trn_rl_repo")

import numpy as np
import ml_dtypes

N = 8192
D = 128
NCORES = 8
RPC = N // NCORES   # 1024 rows per core
NCHUNK = N // 128   # 64 contraction chunks
NPAIR = NCHUNK // 2  # 32 DoubleRow chunk-pairs
NLO = 32            # chunks receiving the lo correction
NLO_PAIR = NLO // 2
NWIN = RPC // 512   # 2 row windows per core
# bulk adjacency supertiles on the sync queue (chunks 0-59); the final 4
# chunks ("tail" supertile) ride the scalar queue EARLY so their completion
# is not gated by the slowest DMA engine draining the whole bulk queue.
SUPERS = [4, 8, 8, 8, 8, 8, 8, 8]       # sync-queue supertiles, chunks 0-59
TAIL_C0, TAIL_SZ = 60, 4                # early scalar-queue supertile
# packed x pieces: (hi chunk range, lo chunk range); xp4 rides scalar early
XP_HI = [(0, 8), (8, 16), (16, 32), (32, 48), (48, 64)]
XP_LO = [(0, 8), (8, 16), (16, 32), (32, 32), (32, 32)]
XP_LENS = [(h1 - h0) + (l1 - l0)
           for (h0, h1), (l0, l1) in zip(XP_HI, XP_LO)]
XP_STARTS = [sum(XP_LENS[:i]) for i in range(len(XP_LENS))]
XTOT = sum(XP_LENS)  # 96 chunks
# sync-queue x pieces land before these chunk offsets (xp4 is on scalar)
XP_BEFORE_CHUNK = [0, 4, 12, 28, None]
N_WARM = 9          # dummy matmuls to ramp the PE clock during DMA wait
S_LO = 64.0         # scale for the lo fp8 plane
BF16 = ml_dtypes.bfloat16
FP8 = ml_dtypes.float8_e4m3fn

_CACHE = {}


def _build_nc():
    from concourse import bacc, bass, tile, mybir

    adt = mybir.dt.float8e4

    nc = bacc.Bacc("TRN2", target_bir_lowering=False, debug=False,
                   num_devices=NCORES)

    adjt_ext = nc.declare_dram_parameter(
        "adjT", [128, NCHUNK, RPC], adt, isOutput=False)
    xc_ext = nc.declare_dram_parameter(
        "xc", [128, XTOT, D], adt, isOutput=False)
    wt_ext = nc.declare_dram_parameter(
        "wT", [D, D], mybir.dt.bfloat16, isOutput=False)
    wt64_ext = nc.declare_dram_parameter(
        "wT64", [D, D], mybir.dt.bfloat16, isOutput=False)
    bb_ext = nc.declare_dram_parameter(
        "bb", [128, D], mybir.dt.float32, isOutput=False)
    dr_ext = nc.declare_dram_parameter(
        "dr", [128, RPC // 128], mybir.dt.float32, isOutput=False)
    out_ext = nc.declare_dram_parameter(
        "out", [128, RPC // 128, D], mybir.dt.bfloat16, isOutput=True)

    DR = mybir.MatmulPerfMode.DoubleRow

    with tile.TileContext(nc) as tc:
        with (
            tc.tile_pool(name="const", bufs=1) as constp,
            tc.tile_pool(name="adj", bufs=1) as adjp,
            tc.tile_pool(name="yt", bufs=2) as ytp,
            tc.tile_pool(name="ot", bufs=2) as otp,
            tc.tile_pool(name="ps_y", bufs=1, space=bass.MemorySpace.PSUM) as psy,
            tc.tile_pool(name="ps_o", bufs=2, space=bass.MemorySpace.PSUM) as pso,
        ):
            xp = [constp.tile([128, XP_LENS[i], D], adt, name=f"xp{i}",
                              tag=f"xp{i}") for i in range(len(XP_LENS))]

            def load_xpiece(i):
                nc.sync.dma_start(
                    out=xp[i][:],
                    in_=xc_ext[:, XP_STARTS[i]:XP_STARTS[i] + XP_LENS[i], :])

            def xsl_hi(q):
                c0 = 2 * q
                i = next(k for k, (h0, h1) in enumerate(XP_HI)
                         if h0 <= c0 < h1)
                o = c0 - XP_HI[i][0]
                return xp[i][:, o:o + 2, :]

            def xsl_lo(q):
                c0 = 2 * q
                i = next(k for k, (l0, l1) in enumerate(XP_LO)
                         if l0 <= c0 < l1)
                o = (XP_HI[i][1] - XP_HI[i][0]) + (c0 - XP_LO[i][0])
                return xp[i][:, o:o + 2, :]

            wt = constp.tile([D, D], mybir.dt.bfloat16, name="wt")
            nc.scalar.dma_start(out=wt[:], in_=wt_ext[:])
            wt64 = constp.tile([D, D], mybir.dt.bfloat16, name="wt64")
            nc.scalar.dma_start(out=wt64[:], in_=wt64_ext[:])
            bb = constp.tile([128, D], mybir.dt.float32, name="bb")
            nc.scalar.dma_start(out=bb[:], in_=bb_ext[:])
            dr = constp.tile([128, RPC // 128], mybir.dt.float32, name="dr")
            nc.scalar.dma_start(out=dr[:], in_=dr_ext[:])

            ps_hi = [psy.tile([128, 512], mybir.dt.float32, tag=f"pshi{w}",
                              name=f"ps_hi{w}") for w in range(NWIN)]
            ps_lo = [psy.tile([128, 512], mybir.dt.float32, tag=f"pslo{w}",
                              name=f"ps_lo{w}") for w in range(NWIN)]

            # warmup: ramp the PE clock out of its low p-state while the
            # first DMAs are in flight (the real stream then runs full speed)
            scr = constp.tile([128, 512], adt, name="warm_src")
            nc.gpsimd.memset(scr[:], 0)
            ps_w = psy.tile([128, 512], mybir.dt.float32, tag="pswarm",
                            name="ps_warm")
            for _ in range(N_WARM):
                nc.tensor.matmul(ps_w[:], lhsT=scr[:, :128], rhs=scr[:],
                                 start=True, stop=True)

            # tail supertile (last 4 chunks) + its x piece on the scalar
            # queue, issued up-front: they complete mid-run, so the final
            # accumulation is gated by st7, not by the bulk queue's last
            # bytes draining on the slowest DMA engine.
            nc.scalar.dma_start(
                out=xp[4][:],
                in_=xc_ext[:, XP_STARTS[4]:XP_STARTS[4] + XP_LENS[4], :])
            at_tail = adjp.tile([128, TAIL_SZ, RPC], adt, tag="adjtail",
                                name="adjtail")
            nc.scalar.dma_start(
                out=at_tail[:], in_=adjt_ext[:, TAIL_C0:TAIL_C0 + TAIL_SZ, :])

            def mm(q, j, w, at, start, stop):
                cs = slice(2 * j, 2 * j + 2)
                ws = slice(w * 512, (w + 1) * 512)
                nc.tensor.matmul(
                    ps_hi[w][:],
                    lhsT=xsl_hi(q),
                    rhs=at[:, cs, ws],
                    start=start,
                    stop=stop,
                    perf_mode=DR,
                )
                if q < NLO_PAIR:
                    nc.tensor.matmul(
                        ps_lo[w][:],
                        lhsT=xsl_lo(q),
                        rhs=at[:, cs, ws],
                        start=start,
                        stop=(q == NLO_PAIR - 1),
                        perf_mode=DR,
                    )

            c0 = 0
            for s, sz in enumerate(SUPERS):
                for i, bc in enumerate(XP_BEFORE_CHUNK):
                    if bc == c0:
                        load_xpiece(i)
                at = adjp.tile([128, sz, RPC], adt, tag=f"adj{s}",
                               name=f"adj{s}")
                nc.sync.dma_start(
                    out=at[:], in_=adjt_ext[:, c0:c0 + sz, :])
                q0 = c0 // 2
                if s < len(SUPERS) - 1:
                    for j in range(sz // 2):
                        for w in range(NWIN):
                            mm(q0 + j, j, w, at, start=(q0 + j == 0),
                               stop=False)
                else:
                    # last emitted supertile window-major: window 0 finishes
                    # early so its epilogue overlaps window 1's tail matmuls
                    for w in range(NWIN):
                        for j in range(sz // 2):
                            mm(q0 + j, j, w, at, start=False,
                               stop=(j == sz // 2 - 1))
                c0 += sz
                if s == 2:
                    # tail chunks (arrived early on the scalar queue) are
                    # accumulated mid-stream; PSUM order is free
                    qt = TAIL_C0 // 2
                    for j in range(TAIL_SZ // 2):
                        for w in range(NWIN):
                            mm(qt + j, j, w, at_tail, start=False,
                               stop=False)

            # epilogue: lo copies unblock early (lo accumulation stops at
            # chunk 32); yh copies split across vector/scalar so both
            # windows' chains run in parallel after the final matmuls
            yl0 = ytp.tile([128, 512], mybir.dt.bfloat16, tag="yl0")
            nc.scalar.copy(yl0[:], ps_lo[0][:])
            yl1 = ytp.tile([128, 512], mybir.dt.bfloat16, tag="yl1")
            nc.scalar.copy(yl1[:], ps_lo[1][:])
            yls = [yl0, yl1]
            for w in range(NWIN):
                yh = ytp.tile([128, 512], mybir.dt.bfloat16, tag=f"yh{w}")
                if w == 0:
                    nc.vector.tensor_copy(yh[:], ps_hi[w][:])
                else:
                    nc.scalar.copy(yh[:], ps_hi[w][:])
                yl = yls[w]
                ot = otp.tile([128, 4, D], mybir.dt.bfloat16, tag="outtile")
                for m in range(4):
                    g = w * 4 + m
                    sl = slice(m * 128, (m + 1) * 128)
                    ps_o = pso.tile([128, D], mybir.dt.float32)
                    nc.tensor.matmul(
                        ps_o[:],
                        lhsT=yh[:, sl],
                        rhs=wt[:],
                        start=True,
                        stop=False,
                    )
                    nc.tensor.matmul(
                        ps_o[:],
                        lhsT=yl[:, sl],
                        rhs=wt64[:],
                        start=False,
                        stop=True,
                    )
                    nc.vector.scalar_tensor_tensor(
                        out=ot[:, m, :],
                        in0=ps_o[:],
                        scalar=dr[:, g:g + 1],
                        in1=bb[:],
                        op0=mybir.AluOpType.mult,
                        op1=mybir.AluOpType.add,
                    )
                nc.scalar.dma_start(out=out_ext[:, w * 4:(w + 1) * 4, :],
                                    in_=ot[:])
    nc.compile()
    return nc


def _host_prep(x, edge_index, W, b):
    r = np.asarray(edge_index[0]).astype(np.int64)
    c = np.asarray(edge_index[1]).astype(np.int64)
    uniq = np.unique(r * N + c)
    r_u = uniq // N
    c_u = uniq % N

    degree = np.bincount(r_u, minlength=N).astype(np.float64) + 1.0
    d = (1.0 / np.sqrt(degree)).astype(np.float32)

    xp = np.asarray(x, dtype=np.float32) * d[:, None]
    xh8 = xp.astype(FP8)
    lo = xp - xh8.astype(np.float32)
    xl8 = (lo * S_LO).astype(FP8)

    # permute the column space so the columns with the largest fp8
    # quantization-error energy land in the corrected chunk range [0, NLO)
    order = np.argsort(-(lo * lo).sum(axis=1), kind="stable")
    P = np.empty(N, dtype=np.int64)
    P[order] = np.arange(N)

    def to_chunks(a, nchunk):
        return a.reshape(nchunk, 128, D).transpose(1, 0, 2)  # [128, chk, feat]

    xh_c = to_chunks(xh8[order], NCHUNK)
    xl_c = to_chunks(xl8[order[:NLO * 128]], NLO)
    # packed pieces: [hi range | lo range] per XP_HI/XP_LO (must match device)
    parts = []
    for (h0, h1), (l0, l1) in zip(XP_HI, XP_LO):
        parts.append(xh_c[:, h0:h1])
        if l1 > l0:
            parts.append(xl_c[:, l0:l1])
    xc = np.ascontiguousarray(np.concatenate(parts, axis=1))

    wt = np.ascontiguousarray(np.asarray(W, dtype=np.float32).T).astype(BF16)
    wt64 = np.ascontiguousarray(
        np.asarray(W, dtype=np.float32).T / S_LO).astype(BF16)
    bb = np.ascontiguousarray(
        np.tile(np.asarray(b, dtype=np.float32)[None, :], (128, 1)))

    in_maps = []
    for k in range(NCORES):
        mask = (r_u // RPC) == k
        rr = r_u[mask] - k * RPC  # local row in [0, RPC)
        cs = P[c_u[mask]]         # permuted global col in [0, N)
        adjt = np.zeros((128, NCHUNK, RPC), dtype=FP8)
        # adjt[p, cc, q] corresponds to adj[row = q (local), col = cc*128+p]
        adjt[cs & 127, cs >> 7, rr] = 1.0
        jj = np.arange(RPC)
        ii = P[k * RPC + jj]  # permuted diag index -> column
        adjt[ii & 127, ii >> 7, jj] += np.ones(RPC, dtype=FP8)
        dr = np.ascontiguousarray(
            d[k * RPC:(k + 1) * RPC].reshape(RPC // 128, 128).T)
        in_maps.append({"adjT": adjt, "xc": xc,
                        "wT": wt, "wT64": wt64, "bb": bb, "dr": dr})
    return in_maps


def _gather(res):
    outs = []
    for k in range(NCORES):
        o = np.asarray(res.results[k]["out"])  # [128, RPC//128, D] bf16
        outs.append(o.transpose(1, 0, 2).reshape(RPC, D))
    return np.ascontiguousarray(np.concatenate(outs, axis=0).astype(np.float32))


def kernel(x, edge_index, W, b):
    from concourse.bass_utils import run_bass_kernel_spmd

    in_maps = _host_prep(x, edge_index, W, b)
    if "nc" not in _CACHE:
        _CACHE["nc"] = _build_nc()
    nc = _CACHE["nc"]
    res = run_bass_kernel_spmd(nc, in_maps, core_ids=list(range(NCORES)))
    return _gather(res)


if __name__ == "__main__":
    rng = np.random.default_rng(0)
    x = rng.standard_normal((N, D), dtype=np.float32)
    ei = rng.integers(0, N, size=(2, 262144)).astype(np.int64)
    W = rng.standard_normal((D, D), dtype=np.float32) / np.sqrt(D)
    b = rng.standard_normal(D, dtype=np.float32) * 0.01
    out = kernel(x=x, edge_index=ei, W=W, b=b)
    print(out.shape, out.dtype, float(np.abs(out).mean()))
